# revision 6
# baseline (speedup 1.0000x reference)
"""Trainium2 Bass kernel for a 3-layer decoder (self-attn + cross-attn + FFN
per layer, final LN, lm_head, fused softmax cross-entropy).

Sharding (8 cores): data-parallel over B (2 groups of 4 cores) x
sequence-parallel within each batch (each core owns 2 token blocks of 128,
blocks {r, 7-r} for load-balanced causal attention). Layer weights are
replicated (streamed from HBM); attention K/V are AllGathered within each
4-core group. The lm_head is vocab-parallel (each core owns V/8 columns) over
all B*T rows, using an AllGathered final hidden state and an AllReduced
softmax statistic for the fused cross-entropy.

The residual stream is fp32; matmuls run in bf16 with fp32 PSUM accumulation.
"""

import numpy as np
import ml_dtypes

BF16 = ml_dtypes.bfloat16

FULL_CFG = dict(V=32000, D=1024, H=16, T=1024, B=2, NL=3, FF=4096)

NC_CORES = 8
GS = 4  # cores per batch element
P = 128

_BUILD_CACHE = {}


def _derive(cfg):
    V, D, H, T, B, NL, FF = (cfg[k] for k in ("V", "D", "H", "T", "B", "NL", "FF"))
    d = dict(cfg)
    d["DH"] = D // H
    d["DT"] = D // P
    d["FT"] = FF // P
    d["TPB"] = T // P                      # token blocks per batch element
    d["NB"] = (B * T) // (P * NC_CORES)    # token blocks per core
    d["TPC"] = d["NB"] * P                 # tokens per core
    d["QB"] = d["NB"]
    d["VS"] = V // NC_CORES
    d["DS"] = min(512, D)
    d["NDS"] = D // d["DS"]
    vss = d["VS"]
    if vss > 500:
        for s in range(500, 0, -1):
            if d["VS"] % s == 0:
                vss = s
                break
    d["VSS"] = vss
    d["NVS"] = d["VS"] // vss
    d["NTT"] = (B * T) // P
    assert d["NB"] in (1, 2)
    return d


def _block_ids(r, TPB, NB):
    return [r] if NB == 1 else [r, TPB - 1 - r]


def _gb_to_chunk(gb, TPB, NB):
    g, j = gb // TPB, gb % TPB
    if NB == 1:
        return g * GS + j, 0
    r = j if j < GS else TPB - 1 - j
    half = 0 if j < GS else 1
    return g * GS + r, half


def _kb_to_chunk(kb, TPB, NB):
    if NB == 1:
        return kb, 0
    r = kb if kb < GS else TPB - 1 - kb
    half = 0 if kb < GS else 1
    return r, half


def _nk_of_q(q_i, TPB):
    return GS if q_i == 0 else TPB


def _mask_kb_lo(q_i):
    return 0 if q_i == 0 else GS


def build(cfg, flags):
    import concourse.bass as bass
    import concourse.mybir as mybir
    import concourse.tile as tile
    from concourse import bacc
    from concourse.masks import make_identity
    from contextlib import ExitStack

    c = _derive(cfg)
    V, D, H, T, B, NL, FF = (c[k] for k in ("V", "D", "H", "T", "B", "NL", "FF"))
    DH, DT, FT, TPB, NB, TPC, QB, VS, VSS, NVS, NTT, DS, NDS = (
        c[k] for k in ("DH", "DT", "FT", "TPB", "NB", "TPC", "QB", "VS", "VSS",
                       "NVS", "NTT", "DS", "NDS"))
    f32 = mybir.dt.float32
    bf = mybir.dt.bfloat16
    i32 = mybir.dt.int32
    AF = mybir.ActivationFunctionType
    ALU = mybir.AluOpType
    KB = TPB
    HPT = P // DH          # heads per partition tile
    HT = H // HPT
    VAW = DH + 1           # V columns per head incl. appended ones column
    groups4 = [[0, 1, 2, 3], [4, 5, 6, 7]]
    groups8 = [list(range(NC_CORES))]
    SELF_SZ = D * TPC
    EPS = 1e-5

    nc = bacc.Bacc("TRN2", debug=False, num_devices=NC_CORES)

    # ---------------- kernel I/O ----------------
    ein = lambda n, s, dt: nc.dram_tensor(n, s, dt, kind="ExternalInput")
    x0_d = ein("x0", [TPC, D], f32)
    kT_d = ein("kT", [D, TPC], bf)
    vT_d = ein("vT", [D, TPC], bf)
    Wq_d = ein("Wq", [NL, D, D], bf)
    Wk_d = ein("Wk", [NL, D, D], bf)
    Wv_d = ein("Wv", [NL, D, D], bf)
    Wo_d = ein("Wo", [NL, D, D], bf)
    Wqc_d = ein("Wqc", [NL, D, D], bf)
    Wkc_d = ein("Wkc", [NL, D, D], bf)
    Wvc_d = ein("Wvc", [NL, D, D], bf)
    Woc_d = ein("Woc", [NL, D, D], bf)
    W1_d = ein("W1", [NL, D, FF], bf)
    W2_d = ein("W2", [NL, FF, D], bf)
    lmW_d = ein("lmW", [D, VS], bf)
    b1_d = ein("b1c", [NL * FT, P], f32)
    pen_d = ein("penT", [QB, KB, P, P], f32)
    toff_d = ein("toff", [NTT, P], i32)
    tmask_d = ein("tmask", [NTT, P], f32)
    if flags["ln_gb"]:
        lngb_d = ein("lngb", [NL * 3 + 1, 2, P, D], f32)
    if flags["b2"]:
        b2_d = ein("b2rep", [NL, P, D], f32)
    if flags["lmb"]:
        lmb_d = ein("lmbrep", [P, VS], f32)

    logits_d = nc.dram_tensor("logits", [B * T, VS], f32, kind="ExternalOutput")
    loss_d = nc.dram_tensor("loss", [1, 1], f32, kind="ExternalOutput")

    with tile.TileContext(nc) as tc:
        top = ExitStack()
        const = top.enter_context(tc.tile_pool(name="const", bufs=1))
        persist = top.enter_context(tc.tile_pool(name="persist", bufs=1))
        dram = top.enter_context(tc.tile_pool(name="dram", bufs=1, space="DRAM"))
        ps = top.enter_context(tc.tile_pool(name="ps", bufs=2, space="PSUM"))

        ident_f = const.tile([P, P], f32)
        make_identity(nc, ident_f)
        ident_b = const.tile([P, P], bf)
        make_identity(nc, ident_b)
        ones_f = const.tile([P, 1], f32)
        nc.vector.memset(ones_f[:], 1.0)

        xres = persist.tile([P, QB * D], f32)
        kTs = persist.tile([P, DT * TPC], bf)
        vTs = persist.tile([P, DT * TPC], bf)
        pen_sb = persist.tile([P, QB * KB * P], f32)
        b1_sb = persist.tile([P, NL * FT], f32)

        for q in range(QB):
            nc.sync.dma_start(xres[:, q * D:(q + 1) * D], x0_d[q * P:(q + 1) * P, :])
        for dt in range(DT):
            nc.sync.dma_start(kTs[:, dt * TPC:(dt + 1) * TPC], kT_d[dt * P:(dt + 1) * P, :])
            nc.sync.dma_start(vTs[:, dt * TPC:(dt + 1) * TPC], vT_d[dt * P:(dt + 1) * P, :])
        for q in range(QB):
            for kb in range(KB):
                nc.sync.dma_start(pen_sb[:, (q * KB + kb) * P:(q * KB + kb + 1) * P],
                                  pen_d[q, kb])
        nc.sync.dma_start(b1_sb[:, :], b1_d[:, :].rearrange("i p -> p i"))
        if flags["b2"]:
            b2_sb = persist.tile([P, NL * D], f32)
            for l in range(NL):
                nc.sync.dma_start(b2_sb[:, l * D:(l + 1) * D], b2_d[l])

        # DRAM scratch for collectives
        cross_bounce = dram.tile([NL, 2, SELF_SZ], bf)
        cross_ag = dram.tile([GS, NL, 2, SELF_SZ], bf)
        self_bounce = dram.tile([2, SELF_SZ], bf)
        self_ag = dram.tile([GS, 2, SELF_SZ], bf)
        xf_bounce = dram.tile([SELF_SZ], bf)
        xf_ag = dram.tile([NC_CORES, SELF_SZ], bf, addr_space="Shared")
        ce_bounce = dram.tile([B * T + P], f32)
        ce_out = dram.tile([B * T + P], f32, addr_space="Shared")

        lyr = ExitStack()
        wp = lyr.enter_context(tc.tile_pool(name="wp", bufs=10))
        w1p = lyr.enter_context(tc.tile_pool(name="w1p", bufs=3))
        w2p = lyr.enter_context(tc.tile_pool(name="w2p", bufs=4))
        hp = lyr.enter_context(tc.tile_pool(name="hp", bufs=2))
        htp = lyr.enter_context(tc.tile_pool(name="htp", bufs=2))
        actp = lyr.enter_context(tc.tile_pool(name="actp", bufs=1))
        kvf = lyr.enter_context(tc.tile_pool(name="kvf", bufs=1))
        ptp = lyr.enter_context(tc.tile_pool(name="ptp", bufs=4))
        h1p = lyr.enter_context(tc.tile_pool(name="h1p", bufs=1))
        stat = lyr.enter_context(tc.tile_pool(name="stat", bufs=4))

        # ---------------- helpers ----------------
        def load_w(src_l, name, n_free):
            tiles = []
            for kt in range(DT):
                t = wp.tile([P, n_free], bf, name=f"{name}_{kt}", tag="w")
                nc.sync.dma_start(t[:], src_l[kt * P:(kt + 1) * P, :])
                tiles.append(t)
            return tiles

        def emit_ln(site, name):
            """LN of xres -> hT feature-major bf16 [P, DT*TPC]."""
            if flags["ln_gb"]:
                lgt = stat.tile([P, D], f32, name=f"lg_{name}", tag="lngain", bufs=2)
                lbt = stat.tile([P, D], f32, name=f"lb_{name}", tag="lnbias", bufs=2)
                nc.sync.dma_start(lgt[:], lngb_d[site, 0])
                nc.sync.dma_start(lbt[:], lngb_d[site, 1])
            hT = htp.tile([P, DT * TPC], bf, name=f"hT_{name}", tag="hT")
            for q in range(QB):
                x_sl = xres[:, q * D:(q + 1) * D]
                nsub = max(1, D // 512)
                st6 = stat.tile([P, nsub, 6], f32, name=f"st6_{name}{q}", tag="st6")
                xv = x_sl.rearrange("p (n s) -> p n s", n=nsub)
                for s in range(nsub):
                    nc.vector.bn_stats(st6[:, s, :], xv[:, s, :])
                mv = stat.tile([P, 2], f32, name=f"mv_{name}{q}", tag="mv")
                nc.vector.bn_aggr(mv[:], st6[:])
                veps = stat.tile([P, 1], f32, name=f"ve_{name}{q}", tag="ve")
                nc.vector.tensor_scalar_add(veps[:], mv[:, 1:2], EPS)
                sd = stat.tile([P, 1], f32, name=f"sd_{name}{q}", tag="sd")
                nc.scalar.sqrt(sd[:], veps[:])
                rstd = stat.tile([P, 1], f32, name=f"rs_{name}{q}", tag="rs")
                nc.vector.reciprocal(rstd[:], sd[:])
                nmr = stat.tile([P, 1], f32, name=f"nm_{name}{q}", tag="nm")
                nc.vector.scalar_tensor_tensor(
                    nmr[:], mv[:, 0:1], -1.0, rstd[:], ALU.mult, ALU.mult)
                h = hp.tile([P, D], f32, name=f"h_{name}{q}", tag="h")
                nc.vector.tensor_scalar(h[:], x_sl, rstd[:], nmr[:], ALU.mult, ALU.add)
                if flags["ln_gb"]:
                    nc.vector.tensor_tensor(h[:], h[:], lgt[:], ALU.mult)
                    nc.vector.tensor_tensor(h[:], h[:], lbt[:], ALU.add)
                for dt in range(DT):
                    pst = ps.tile([P, P], f32, name=f"pst_{name}{q}{dt}", tag="tp", bufs=2)
                    nc.tensor.transpose(pst[:], h[:, dt * P:(dt + 1) * P], ident_f[:])
                    nc.any.tensor_copy(hT[:, dt * TPC + q * P: dt * TPC + (q + 1) * P],
                                       pst[:])
            return hT

        def proj_featmaj(hT_sb, w_tiles, name, tag, scale=None):
            """out^T = W^T @ h^T : feature-major [P, DT*TPC] bf16."""
            oT = actp.tile([P, DT * TPC], bf, name=f"oT_{name}", tag=tag)
            for dt in range(DT):
                psb = ps.tile([P, TPC], f32, name=f"ps_{name}{dt}", tag="psb", bufs=2)
                for kt in range(DT):
                    nc.tensor.matmul(
                        psb[:], lhsT=w_tiles[kt][:, dt * P:(dt + 1) * P],
                        rhs=hT_sb[:, kt * TPC:(kt + 1) * TPC],
                        start=(kt == 0), stop=(kt == DT - 1))
                dst = oT[:, dt * TPC:(dt + 1) * TPC]
                if scale is not None:
                    nc.scalar.mul(dst, psb[:], scale)
                else:
                    nc.any.tensor_copy(dst, psb[:])
            return oT

        def proj_tokmaj(hT_sb, w_tiles, name, tag):
            """out = h @ W : token-major [P, QB*D] bf16."""
            o = actp.tile([P, QB * D], bf, name=f"o_{name}", tag=tag)
            for q in range(QB):
                for ns in range(NDS):
                    psb = ps.tile([P, DS], f32, name=f"ps_{name}{q}{ns}", tag="psb", bufs=2)
                    for kt in range(DT):
                        nc.tensor.matmul(
                            psb[:], lhsT=hT_sb[:, kt * TPC + q * P: kt * TPC + (q + 1) * P],
                            rhs=w_tiles[kt][:, ns * DS:(ns + 1) * DS],
                            start=(kt == 0), stop=(kt == DT - 1))
                    nc.any.tensor_copy(o[:, q * D + ns * DS: q * D + (ns + 1) * DS], psb[:])
            return o

        def proj_tokmaj_residual(aT_sb, w_tiles, name, bias_sb=None, bias_off=0):
            """xres += a @ W (+ bias).  aT_sb feature-major [P, DT*TPC] bf16."""
            for q in range(QB):
                for ns in range(NDS):
                    psb = ps.tile([P, DS], f32, name=f"ps_{name}{q}{ns}", tag="psb", bufs=2)
                    nkt = len(aT_sb) if isinstance(aT_sb, list) else DT
                    for kt in range(DT):
                        nc.tensor.matmul(
                            psb[:], lhsT=aT_sb[:, kt * TPC + q * P: kt * TPC + (q + 1) * P],
                            rhs=w_tiles[kt][:, ns * DS:(ns + 1) * DS],
                            start=(kt == 0), stop=(kt == DT - 1))
                    x_sl = xres[:, q * D + ns * DS: q * D + (ns + 1) * DS]
                    if bias_sb is not None:
                        nc.vector.tensor_tensor(
                            psb[:], psb[:],
                            bias_sb[:, bias_off + ns * DS: bias_off + (ns + 1) * DS],
                            ALU.add)
                    nc.vector.tensor_tensor(x_sl, x_sl, psb[:], ALU.add)

        def transpose_to_featmaj(a_sb, name):
            """token-major [P, QB*D] bf16 -> feature-major [P, DT*TPC] bf16."""
            aT = actp.tile([P, DT * TPC], bf, name=f"aT_{name}", tag="aT")
            for q in range(QB):
                for dt in range(DT):
                    pst = ps.tile([P, P], bf, name=f"pt_{name}{q}{dt}", tag="tp", bufs=2)
                    nc.tensor.transpose(
                        pst[:], a_sb[:, q * D + dt * P: q * D + (dt + 1) * P], ident_b[:])
                    nc.any.tensor_copy(aT[:, dt * TPC + q * P: dt * TPC + (q + 1) * P],
                                       pst[:])
            return aT

        def attention(qT_sb, ktf, vaug, name, causal):
            """qT_sb: pre-scaled q^T feature-major. ktf: K^T full [P, DT*KB*P].
            vaug: [P, KB*H*VAW] token-major V with ones column per head.
            Returns ao token-major [P, QB*D] bf16."""
            ao = actp.tile([P, QB * D], bf, name=f"ao_{name}", tag="ao")
            for h in range(H):
                ht_i, off = h // HPT, (h % HPT) * DH
                for q in range(QB):
                    nk = _nk_of_q(q, TPB) if causal else KB
                    mlo = _mask_kb_lo(q) if causal else KB
                    avp = ps.tile([P, VAW], f32, name=f"av_{name}{h}{q}", tag="av", bufs=2)
                    for kb in range(nk):
                        stp = ps.tile([P, P], f32, name=f"st_{name}{h}{q}{kb}",
                                      tag="st", bufs=2)
                        nc.tensor.matmul(
                            stp[:],
                            lhsT=ktf[off:off + DH,
                                     ht_i * (KB * P) + kb * P: ht_i * (KB * P) + (kb + 1) * P],
                            rhs=qT_sb[off:off + DH,
                                      ht_i * TPC + q * P: ht_i * TPC + (q + 1) * P],
                            start=True, stop=True)
                        if causal and kb >= mlo:
                            nc.vector.tensor_tensor(
                                stp[:], stp[:],
                                pen_sb[:, (q * KB + kb) * P:(q * KB + kb + 1) * P],
                                ALU.add)
                        pt = ptp.tile([P, P], bf, name=f"p_{name}{h}{q}{kb}", tag="pt")
                        nc.scalar.activation(pt[:], stp[:], AF.Exp)
                        nc.tensor.matmul(
                            avp[:], lhsT=pt[:],
                            rhs=vaug[:, kb * (H * VAW) + h * VAW: kb * (H * VAW) + (h + 1) * VAW],
                            start=(kb == 0), stop=(kb == nk - 1))
                    rinv = stat.tile([P, 1], f32, name=f"ri_{name}{h}{q}", tag="ri")
                    nc.vector.reciprocal(rinv[:], avp[:, DH:DH + 1])
                    nc.vector.tensor_scalar_mul(
                        ao[:, q * D + h * DH: q * D + (h + 1) * DH],
                        avp[:, 0:DH], rinv[:])
            return ao

        def build_kv_full(ag_ap, name):
            ktf = kvf.tile([P, DT * KB * P], bf, name=f"ktf_{name}", tag="ktf")
            vaug = kvf.tile([P, KB * H * VAW], bf, name=f"vaug_{name}", tag="vaug")
            for kb in range(KB):
                ch, half = _kb_to_chunk(kb, TPB, NB)
                kt_src = ag_ap(ch, 0).rearrange("(d p c) -> d p c", d=DT, p=P)
                for dt in range(DT):
                    nc.sync.dma_start(
                        ktf[:, dt * (KB * P) + kb * P: dt * (KB * P) + (kb + 1) * P],
                        kt_src[dt, :, half * P:(half + 1) * P])
                v_src = ag_ap(ch, 1).rearrange("(q p c) -> q p c", q=NB, p=P)
                dst3 = vaug[:, kb * (H * VAW):(kb + 1) * (H * VAW)].rearrange(
                    "p (h w) -> p h w", h=H)
                nc.sync.dma_start(dst3[:, :, 0:DH],
                                  v_src[half].rearrange("p (h d) -> p h d", h=H))
                nc.vector.memset(dst3[:, :, DH:DH + 1], 1.0)
            return ktf, vaug

        # ---------------- P1: cross K/V for all layers, one AllGather -------
        for l in range(NL):
            wkc = load_w(Wkc_d[l], f"wkc{l}", D)
            kcT = proj_featmaj(kTs, wkc, f"kcT{l}", tag="kc")
            cb_k = cross_bounce[l, 0].rearrange("(d p c) -> d p c", d=DT, p=P)
            for dt in range(DT):
                nc.sync.dma_start(cb_k[dt], kcT[:, dt * TPC:(dt + 1) * TPC])
            wvc = load_w(Wvc_d[l], f"wvc{l}", D)
            vc = proj_tokmaj(vTs, wvc, f"vc{l}", tag="vc")
            cb_v = cross_bounce[l, 1].rearrange("(q p c) -> q p c", q=NB, p=P)
            for q in range(NB):
                nc.sync.dma_start(cb_v[q], vc[:, q * D:(q + 1) * D])
        nc.gpsimd.collective_compute(
            "AllGather", ALU.bypass, replica_groups=groups4,
            ins=[cross_bounce[:].rearrange("a b c -> (a b c)")],
            outs=[cross_ag[:].rearrange("g a b c -> (g a b c)")])

        # ---------------- P2: transformer layers ----------------
        for l in range(NL):
            # ---- self attention ----
            hT = emit_ln(l * 3 + 0, f"ln1_{l}")
            wq = load_w(Wq_d[l], f"wq{l}", D)
            qT = proj_featmaj(hT, wq, f"qT{l}", tag="qT", scale=1.0 / float(np.sqrt(DH)))
            wk = load_w(Wk_d[l], f"wk{l}", D)
            kT_l = proj_featmaj(hT, wk, f"kTl{l}", tag="kT")
            sb_k = self_bounce[0].rearrange("(d p c) -> d p c", d=DT, p=P)
            for dt in range(DT):
                nc.sync.dma_start(sb_k[dt], kT_l[:, dt * TPC:(dt + 1) * TPC])
            wv = load_w(Wv_d[l], f"wv{l}", D)
            v_l = proj_tokmaj(hT, wv, f"vl{l}", tag="vl")
            sb_v = self_bounce[1].rearrange("(q p c) -> q p c", q=NB, p=P)
            for q in range(NB):
                nc.sync.dma_start(sb_v[q], v_l[:, q * D:(q + 1) * D])
            nc.gpsimd.collective_compute(
                "AllGather", ALU.bypass, replica_groups=groups4,
                ins=[self_bounce[:].rearrange("a b -> (a b)")],
                outs=[self_ag[:].rearrange("g a b -> (g a b)")])
            ktf, vaug = build_kv_full(lambda ch, w: self_ag[ch, w], f"s{l}")
            ao = attention(qT, ktf, vaug, f"sa{l}", causal=True)
            aoT = transpose_to_featmaj(ao, f"sao{l}")
            wo = load_w(Wo_d[l], f"wo{l}", D)
            proj_tokmaj_residual(aoT, wo, f"wo{l}")

            # ---- cross attention ----
            hT = emit_ln(l * 3 + 1, f"ln2_{l}")
            wqc = load_w(Wqc_d[l], f"wqc{l}", D)
            qcT = proj_featmaj(hT, wqc, f"qcT{l}", tag="qT", scale=1.0 / float(np.sqrt(DH)))
            ktfc, vaugc = build_kv_full(lambda ch, w: cross_ag[ch, l, w], f"c{l}")
            aoc = attention(qcT, ktfc, vaugc, f"ca{l}", causal=False)
            aocT = transpose_to_featmaj(aoc, f"cao{l}")
            woc = load_w(Woc_d[l], f"woc{l}", D)
            proj_tokmaj_residual(aocT, woc, f"woc{l}")

            # ---- FFN ----
            hT = emit_ln(l * 3 + 2, f"ln3_{l}")
            h1T = h1p.tile([P, FT * TPC], bf, name=f"h1T_{l}", tag="h1T")
            for ft in range(FT):
                w1b = w1p.tile([P, DT * P], bf, name=f"w1b_{l}{ft}", tag="w1b")
                nc.sync.dma_start(
                    w1b[:].rearrange("p (kt c) -> p kt c", kt=DT),
                    W1_d[l][:, ft * P:(ft + 1) * P].rearrange("(kt p) c -> p kt c", p=P))
                psb = ps.tile([P, TPC], f32, name=f"psf_{l}{ft}", tag="psb", bufs=2)
                for kt in range(DT):
                    nc.tensor.matmul(
                        psb[:], lhsT=w1b[:, kt * P:(kt + 1) * P],
                        rhs=hT[:, kt * TPC:(kt + 1) * TPC],
                        start=(kt == 0), stop=(kt == DT - 1))
                nc.scalar.activation(h1T[:, ft * TPC:(ft + 1) * TPC], psb[:],
                                     AF.Relu, bias=b1_sb[:, l * FT + ft: l * FT + ft + 1])
            for ns in range(NDS):
                psq = [ps.tile([P, DS], f32, name=f"psw2_{l}{q}{ns}", tag="psb", bufs=2)
                       for q in range(QB)]
                for kt in range(FT):
                    w2b = w2p.tile([P, DS], bf, name=f"w2b_{l}{ns}{kt}", tag="w2b")
                    nc.sync.dma_start(w2b[:], W2_d[l, kt * P:(kt + 1) * P,
                                                   ns * DS:(ns + 1) * DS])
                    for q in range(QB):
                        nc.tensor.matmul(
                            psq[q][:], lhsT=h1T[:, kt * TPC + q * P: kt * TPC + (q + 1) * P],
                            rhs=w2b[:], start=(kt == 0), stop=(kt == FT - 1))
                for q in range(QB):
                    x_sl = xres[:, q * D + ns * DS: q * D + (ns + 1) * DS]
                    if flags["b2"]:
                        nc.vector.tensor_tensor(
                            psq[q][:], psq[q][:],
                            b2_sb[:, l * D + ns * DS: l * D + (ns + 1) * DS], ALU.add)
                    nc.vector.tensor_tensor(x_sl, x_sl, psq[q][:], ALU.add)

        # ---------------- P3: final LN + hidden-state AllGather -------------
        xfT = emit_ln(NL * 3, "lnf")
        xb = xf_bounce[:].rearrange("(d p c) -> d p c", d=DT, p=P)
        for dt in range(DT):
            nc.sync.dma_start(xb[dt], xfT[:, dt * TPC:(dt + 1) * TPC])
        nc.gpsimd.collective_compute(
            "AllGather", ALU.bypass, replica_groups=groups8,
            ins=[xf_bounce[:]], outs=[xf_ag[:].rearrange("g a -> (g a)")])
        lyr.close()

        # ---------------- P4: lm head + fused cross entropy ----------------
        lmp = top.enter_context(tc.tile_pool(name="lmp", bufs=1))
        lgp = top.enter_context(tc.tile_pool(name="lgp", bufs=2))
        xft_full = lmp.tile([P, DT * B * T], bf)
        for gb in range(NTT):
            ch, half = _gb_to_chunk(gb, TPB, NB)
            src = xf_ag[ch].rearrange("(d p c) -> d p c", d=DT, p=P)
            for dt in range(DT):
                nc.sync.dma_start(
                    xft_full[:, dt * (B * T) + gb * P: dt * (B * T) + (gb + 1) * P],
                    src[dt, :, half * P:(half + 1) * P])
        lmw_sb = lmp.tile([P, DT * VS], bf)
        for dt in range(DT):
            nc.sync.dma_start(lmw_sb[:, dt * VS:(dt + 1) * VS],
                              lmW_d[dt * P:(dt + 1) * P, :])
        if flags["lmb"]:
            lmb_sb = lmp.tile([P, VS], f32)
            nc.sync.dma_start(lmb_sb[:], lmb_d[:, :])
        toff_sb = lmp.tile([P, NTT], i32)
        nc.sync.dma_start(toff_sb[:], toff_d[:, :].rearrange("t p -> p t"))
        tmask_sb = lmp.tile([P, NTT], f32)
        nc.sync.dma_start(tmask_sb[:], tmask_d[:, :].rearrange("t p -> p t"))
        se_all = lmp.tile([P, NTT], f32)

        for tt in range(NTT):
            lg = lgp.tile([P, VS], f32, name=f"lg{tt}", tag="lg")
            for ns in range(NVS):
                psb = ps.tile([P, VSS], f32, name=f"pslm{tt}{ns}", tag="psb", bufs=2)
                for dt in range(DT):
                    nc.tensor.matmul(
                        psb[:],
                        lhsT=xft_full[:, dt * (B * T) + tt * P: dt * (B * T) + (tt + 1) * P],
                        rhs=lmw_sb[:, dt * VS + ns * VSS: dt * VS + (ns + 1) * VSS],
                        start=(dt == 0), stop=(dt == DT - 1))
                nc.any.tensor_copy(lg[:, ns * VSS:(ns + 1) * VSS], psb[:])
            if flags["lmb"]:
                nc.vector.tensor_tensor(lg[:], lg[:], lmb_sb[:], ALU.add)
            nc.sync.dma_start(logits_d[tt * P:(tt + 1) * P, :], lg[:])
            esc = lgp.tile([P, VS], bf, name=f"esc{tt}", tag="esc")
            nc.scalar.activation(esc[:], lg[:], AF.Exp, accum_out=se_all[:, tt:tt + 1])

        # target-logit gather (after all logits rows are in DRAM)
        tacc = lmp.tile([P, 1], f32)
        nc.vector.memset(tacc[:], 0.0)
        lg_flat = logits_d[:, :].rearrange("a (b o) -> (a b) o", o=1)
        for tt in range(NTT):
            tl = lgp.tile([P, 1], f32, name=f"tl{tt}", tag="tl")
            nc.gpsimd.indirect_dma_start(
                out=tl[:], out_offset=None, in_=lg_flat,
                in_offset=bass.IndirectOffsetOnAxis(ap=toff_sb[:, tt:tt + 1], axis=0))
            nc.vector.scalar_tensor_tensor(
                tacc[:], tl[:], tmask_sb[:, tt:tt + 1], tacc[:], ALU.mult, ALU.add)

        # AllReduce [sum_exp per row ; per-partition target-logit sums]
        nc.sync.dma_start(ce_bounce[0:B * T].rearrange("(p t) -> p t", p=P), se_all[:])
        nc.sync.dma_start(ce_bounce[B * T:B * T + P].rearrange("(p o) -> p o", p=P),
                          tacc[:])
        nc.gpsimd.collective_compute(
            "AllReduce", ALU.add, replica_groups=groups8,
            ins=[ce_bounce[:]], outs=[ce_out[:]])
        seg = lmp.tile([P, NTT], f32)
        nc.sync.dma_start(seg[:], ce_out[0:B * T].rearrange("(p t) -> p t", p=P))
        tsg = lmp.tile([P, 1], f32)
        nc.sync.dma_start(tsg[:], ce_out[B * T:B * T + P].rearrange("(p o) -> p o", p=P))
        lnz = lmp.tile([P, NTT], f32)
        lacc = lmp.tile([P, 1], f32)
        nc.scalar.activation(lnz[:], seg[:], AF.Ln, accum_out=lacc[:])
        dvec = lmp.tile([P, 1], f32)
        nc.vector.tensor_sub(dvec[:], lacc[:], tsg[:])
        psl = ps.tile([1, 1], f32, name="psloss", tag="psb", bufs=2)
        nc.tensor.matmul(psl[:], lhsT=dvec[:], rhs=ones_f[:, 0:1], start=True, stop=True)
        loss_sb = lmp.tile([1, 1], f32)
        nc.scalar.mul(loss_sb[:], psl[:], 1.0 / (B * T))
        nc.sync.dma_start(loss_d[:, :], loss_sb[:])

        top.close()

    nc.compile()
    return nc


def prep_inputs(cfg, idx, k, v, targets, params):
    c = _derive(cfg)
    V, D, H, T, B, NL, FF = (c[k2] for k2 in ("V", "D", "H", "T", "B", "NL", "FF"))
    DH, DT, FT, TPB, NB, TPC, QB, VS, NTT = (
        c[k2] for k2 in ("DH", "DT", "FT", "TPB", "NB", "TPC", "QB", "VS", "NTT"))
    KB = TPB
    p = params
    tok_emb = np.asarray(p["tok_emb"], np.float32)
    pos_emb = np.asarray(p["pos_emb"], np.float32)
    idx = np.asarray(idx)
    targets = np.asarray(targets).reshape(-1)
    k = np.asarray(k, np.float32)
    v = np.asarray(v, np.float32)

    wcast = lambda name: np.ascontiguousarray(np.asarray(p[name], np.float32).astype(BF16))
    shared = {n2: wcast(n1) for n1, n2 in [
        ("Wq_s", "Wq"), ("Wk_s", "Wk"), ("Wv_s", "Wv"), ("Wo_s", "Wo"),
        ("Wq_c", "Wqc"), ("Wk_c", "Wkc"), ("Wv_c", "Wvc"), ("Wo_c", "Woc"),
        ("W1", "W1"), ("W2", "W2")]}
    b1 = np.asarray(p["b1"], np.float32)
    shared["b1c"] = np.ascontiguousarray(b1.reshape(NL * FT, P))

    flags = dict(
        ln_gb=not (all(
            np.all(np.asarray(p[f"ln{i}_g"]) == 1) and np.all(np.asarray(p[f"ln{i}_b"]) == 0)
            for i in (1, 2, 3)) and
            np.all(np.asarray(p["lnf_g"]) == 1) and np.all(np.asarray(p["lnf_b"]) == 0)),
        b2=bool(np.any(np.asarray(p["b2"]) != 0)),
        lmb=bool(np.any(np.asarray(p["lm_b"]) != 0)),
    )
    if flags["ln_gb"]:
        lngb = np.zeros((NL * 3 + 1, 2, P, D), np.float32)
        for l in range(NL):
            for s, nm in enumerate(("ln1", "ln2", "ln3")):
                lngb[l * 3 + s, 0, :, :] = np.asarray(p[f"{nm}_g"], np.float32)[l][None, :]
                lngb[l * 3 + s, 1, :, :] = np.asarray(p[f"{nm}_b"], np.float32)[l][None, :]
        lngb[NL * 3, 0, :, :] = np.asarray(p["lnf_g"], np.float32)[None, :]
        lngb[NL * 3, 1, :, :] = np.asarray(p["lnf_b"], np.float32)[None, :]
        shared["lngb"] = np.ascontiguousarray(lngb)
    if flags["b2"]:
        shared["b2rep"] = np.ascontiguousarray(
            np.broadcast_to(np.asarray(p["b2"], np.float32)[:, None, :], (NL, P, D)))

    lm_W = np.asarray(p["lm_W"], np.float32)
    lm_b = np.asarray(p["lm_b"], np.float32)

    in_maps = []
    for core in range(NC_CORES):
        g, r = core // GS, core % GS
        bl = _block_ids(r, TPB, NB)
        rows = np.concatenate([np.arange(j * P, (j + 1) * P) for j in bl])
        x0 = tok_emb[idx[g, rows]] + pos_emb[rows]
        kT = np.ascontiguousarray(k[g, rows, :].T.astype(BF16))
        vT = np.ascontiguousarray(v[g, rows, :].T.astype(BF16))
        pen = np.zeros((QB, KB, P, P), np.float32)
        for q_i, j in enumerate(bl):
            qglob = j * P + np.arange(P)[None, :]
            for kb in range(KB):
                kglob = kb * P + np.arange(P)[:, None]
                pen[q_i, kb] = np.where(kglob <= qglob, 0.0, -1e9)
        tcol = targets - core * VS
        own = (tcol >= 0) & (tcol < VS)
        toff = (np.arange(B * T) * VS + np.clip(tcol, 0, VS - 1)).astype(np.int32)
        toff[~own] = 0
        tmask = own.astype(np.float32)
        m = dict(shared)
        m.update(
            x0=np.ascontiguousarray(x0, dtype=np.float32),
            kT=kT, vT=vT, penT=pen,
            toff=np.ascontiguousarray(toff.reshape(NTT, P)),
            tmask=np.ascontiguousarray(tmask.reshape(NTT, P)),
            lmW=np.ascontiguousarray(lm_W[:, core * VS:(core + 1) * VS].astype(BF16)),
        )
        if flags["lmb"]:
            m["lmbrep"] = np.ascontiguousarray(np.broadcast_to(
                lm_b[None, core * VS:(core + 1) * VS], (P, VS)).astype(np.float32))
        in_maps.append(m)
    return in_maps, flags


def get_built(cfg, flags):
    key = (tuple(sorted(cfg.items())), tuple(sorted(flags.items())))
    if key not in _BUILD_CACHE:
        _BUILD_CACHE[key] = build(cfg, flags)
    return _BUILD_CACHE[key]


def run(cfg, idx, k, v, targets, params, trace=False):
    from concourse.bass_utils import run_bass_kernel_spmd
    in_maps, flags = prep_inputs(cfg, idx, k, v, targets, params)
    nc = get_built(cfg, flags)
    res = run_bass_kernel_spmd(nc, in_maps, core_ids=list(range(NC_CORES)),
                               trace=trace)
    logits = np.concatenate([res.results[c]["logits"] for c in range(NC_CORES)],
                            axis=1)
    loss = np.float32(res.results[0]["loss"][0, 0])
    return (logits, loss), res


def kernel(idx, k, v, targets, params):
    (logits, loss), _ = run(FULL_CFG, idx, k, v, targets, params)
    return logits, loss
